# revision 33
# baseline (speedup 1.0000x reference)
"""Trainium2 Bass kernel for nn_CrossAttentionExpert (bf16 pipeline).

Problem (hardcoded): B=4, C=256, H=W=64 (N=4096), C8=32.
  cross_p2v = attn(q=wq_p@f_p, k=wk_v@f_v, v=wv_v@f_v)
  cross_v2p = attn(q=wq_v@f_v, k=wk_p@f_p, v=wv_p@f_p)
  out = BN(w_out @ concat([f_p, f_v, cross_p2v, cross_v2p]))  (training BN)

Sharding: 8 cores = (batch b, spatial half h); each core computes both
attention directions for its 2048 queries against all 4096 keys of its
batch, plus the fused output conv; BN sum/sumsq are AllReduced ([128,4]
fp32) across the 8 cores.

Layout/speed tricks vs the f32r version (which ran at ~700us):
- everything bf16: 4x-packed K=32 score matmuls via tile_position row
  tiling (kt lives on 4 partition bands, qr replicated to 4 bands by a
  col-tiled Q conv), FWL weight loads, 2x DVE modes, half the DMA bytes.
- softmax rowsum = bf16 add-tree on DVE (tensor_reduce is capped at 1x
  and measured 245us total in the old kernel) + a ones-column PE matmul
  for the final partition fold; 1/rowsum is applied to the 256-channel
  attention output (av) rather than to the NxN probabilities, and the
  V bias is dropped entirely: a per-channel constant shifts every
  position equally, so training-mode BN cancels it exactly.
- BN stats via incremental bn_stats/bn_aggr per m-tile (hidden under
  the attention loop); the sqrt activation table is preloaded during
  the AllReduce wait.
"""

import numpy as np
import ml_dtypes

import concourse.bass as bass
import concourse.mybir as mybir
import concourse.tile as tile
from concourse import bacc, bass_utils

BF = mybir.dt.bfloat16
FP = mybir.dt.float32
P = 128
C = 256
C8 = 32
N = 4096          # keys per core (full spatial positions of its batch)
M = 2048          # queries per core
MT = 512          # m-tile width
NMT = 4
NG = 8            # score groups per m-tile (4 key-chunks of 128 each)
NCORES = 8
BN_EPS = 1e-5
BN_COUNT = 4 * 4096  # B * H * W

_ALU = mybir.AluOpType
_ACT = mybir.ActivationFunctionType

_PROGRAM = None

def _build_program():
    nc = bacc.Bacc("TRN2", target_bir_lowering=False, debug=False,
                   num_devices=NCORES)

    # ---- DRAM I/O ----
    kv = [nc.dram_tensor(f"kv{d}", [C, N], BF, kind="ExternalInput").ap()
          for d in range(2)]
    wq = [nc.dram_tensor(f"wq{d}", [C, C8], BF, kind="ExternalInput").ap()
          for d in range(2)]
    wk = [nc.dram_tensor(f"wk{d}", [C, C8], BF, kind="ExternalInput").ap()
          for d in range(2)]
    wv = [nc.dram_tensor(f"wv{d}", [C, C], BF, kind="ExternalInput").ap()
          for d in range(2)]
    wout = nc.dram_tensor("wout", [4 * C, C], BF, kind="ExternalInput").ap()
    biasq = nc.dram_tensor("biasq", [P, 4], FP, kind="ExternalInput").ap()
    cvec = nc.dram_tensor("cvec", [P, 4], FP, kind="ExternalInput").ap()
    yout = nc.dram_tensor("y", [C, M], BF, kind="ExternalOutput").ap()

    with tile.TileContext(nc) as tc:
        with (
            tc.tile_pool(name="consts", bufs=1) as consts,
            tc.tile_pool(name="big", bufs=1) as big,
            tc.tile_pool(name="vt", bufs=32) as vtp,
            tc.tile_pool(name="stg", bufs=4) as stp,
            tc.tile_pool(name="racc", bufs=4) as p_racc,
            tc.tile_pool(name="tmp", bufs=4) as p_tmp,
            tc.tile_pool(name="rinv", bufs=4) as p_rinv,
            tc.tile_pool(name="rbc", bufs=2) as p_rbc,
            tc.tile_pool(name="avsb", bufs=4) as p_avsb,
            tc.tile_pool(name="small", bufs=8) as p_small,
            tc.tile_pool(name="bn", bufs=1) as p_bn,
            tc.tile_pool(name="psS", bufs=2, space="PSUM") as psS,
            tc.tile_pool(name="psA", bufs=2, space="PSUM") as psA,
            tc.tile_pool(name="psM", bufs=2, space="PSUM") as psM,
            tc.tile_pool(name="dram", bufs=1, space="DRAM") as dram,
        ):
            # ---- load inputs/weights to SBUF ----
            # weights first (tiny, gate everything), then kv quarter-major
            # so the direct/Q/K/V convs can start after the first quarter.
            kv_sb = [big.tile([P, 2, N], BF, name=f"kvsb{d}")
                     for d in range(2)]
            kv_src = [kv[d].rearrange("(o p) n -> p o n", p=P)
                      for d in range(2)]

            def load_kv_q(q):
                sl = slice(q * 1024, (q + 1) * 1024)
                for d in (1, 0):
                    nc.sync.dma_start(kv_sb[d][:, :, sl],
                                      kv_src[d][:, :, sl])

            def load_w(ap, shape, name, dt=BF):
                t = consts.tile(shape, dt, name=name)
                nc.sync.dma_start(
                    t[:], ap.rearrange("(o p) m -> p o m", p=P))
                return t

            # DMA order = first-consumer order: tiny Q/K weights + bias,
            # then the first kv quarter (Q/K convs of dir 0 can start on
            # it), then the rest interleaved, big wout last.
            wq_sb = [load_w(wq[d], [P, 2, C8], f"wqsb{d}") for d in range(2)]
            wk_sb = [load_w(wk[d], [P, 2, C8], f"wksb{d}") for d in range(2)]
            biasq_sb = consts.tile([P, 4], FP, name="biasqsb")
            nc.sync.dma_start(biasq_sb[:], biasq[:])
            load_kv_q(0)
            wv_sb = [load_w(wv[d], [P, 2, C], f"wvsb{d}") for d in range(2)]
            load_kv_q(1)
            wout_sb = load_w(wout, [P, 8, C], "woutsb")
            cvec_sb = consts.tile([P, 4], FP, name="cvecsb")
            nc.sync.dma_start(cvec_sb[:], cvec[:])

            ones_col = consts.tile([P, 1], BF, name="ones_col")
            nc.vector.memset(ones_col[:], 1.0)

            # Warm-up AllReduce during the DMA head: loads the CC-core
            # ucode and start-aligns the 8 cores, so the BN AllReduce at
            # the tail doesn't pay ~12us startup + ~20us peer-skew wait.
            warm_sb = p_small.tile([P, 1], FP, tag="warm")
            nc.vector.memset(warm_sb[:], 0.0)
            warm_in = dram.tile([P, 1], FP)
            warm_out = dram.tile([P, 1], FP)
            nc.sync.dma_start(warm_in[:], warm_sb[:])
            nc.gpsimd.collective_compute(
                "AllReduce", _ALU.add,
                replica_groups=[list(range(NCORES))],
                ins=[warm_in.opt()], outs=[warm_out.opt()])

            load_kv_q(2)
            load_kv_q(3)

            # ---- persistent activations ----
            # qr[d]: Q result replicated on all 4 partition bands, [128, M]
            # kt[d]: K result, band i / free-slot p holds keys of sub 4p+i,
            #        [128, 1024]
            qr = [big.tile([P, M], BF, name=f"qr{d}") for d in range(2)]
            kt = [big.tile([P, 1024], BF, name=f"kt{d}") for d in range(2)]
            y_acc = [big.tile([P, M], FP, name=f"yacc{cc}") for cc in range(2)]
            bnacc = [p_bn.tile([P, NMT, 6], FP, name=f"bnacc{cc}")
                     for cc in range(2)]

            # ---- direct terms of the output conv ----
            # y = wout[:, :256] @ f_p[:, half] + wout[:, 256:512] @ f_v[:, half]
            # (emitted after dir-0's Q/K/V convs: those only need the first
            # kv quarter + tiny weights, so compute starts well before the
            # full 4MB kv load has landed)
            def emit_direct_conv():
                for t in range(NMT):
                    msl = slice(t * MT, (t + 1) * MT)
                    for oc in range(2):
                        ocs = slice(oc * P, (oc + 1) * P)
                        ps = psM.tile([P, MT], FP, tag="misc")
                        nc.tensor.matmul(ps, wout_sb[:, 0, ocs],
                                         kv_sb[1][:, 0, msl],
                                         start=True, stop=False)
                        nc.tensor.matmul(ps, wout_sb[:, 1, ocs],
                                         kv_sb[1][:, 1, msl],
                                         start=False, stop=False)
                        nc.tensor.matmul(ps, wout_sb[:, 2, ocs],
                                         kv_sb[0][:, 0, msl],
                                         start=False, stop=False)
                        nc.tensor.matmul(ps, wout_sb[:, 3, ocs],
                                         kv_sb[0][:, 1, msl],
                                         start=False, stop=True)
                        nc.scalar.copy(y_acc[oc][:, msl], ps)

            # ---- per-direction work ----
            pending = []   # deferred epilogue emit-closures (see m-loop)
            stash = {}
            for d in range(2):
                qkv = kv_sb[1 - d]    # Q source (dir0: f_p=kv1, dir1: f_v)
                kkv = kv_sb[d]        # K/V source

                # Q conv, col-tiled x4 so qr comes out replicated on all
                # 4 partition bands: out psum[32j:32j+32] <- same weights.
                for t in range(NMT):
                    msl = slice(t * MT, (t + 1) * MT)
                    ps = psM.tile([P, MT], FP, tag="misc")
                    for j in range(4):
                        for kc in range(2):
                            nc.tensor.matmul(
                                ps[32 * j:32 * (j + 1), :],
                                wq_sb[d][:, kc, :], qkv[:, kc, msl],
                                start=(kc == 0), stop=(kc == 1),
                                tile_position=(0, 32 * j))
                    nc.vector.tensor_scalar_add(
                        qr[d][:, msl], ps, biasq_sb[:, 2 * d:2 * d + 1])

                # K conv, col-tiled: band i holds keys of sub s=4p+i.
                for p_ in range(2):
                    ps = psM.tile([P, MT], FP, tag="misc")
                    for i in range(4):
                        s = 4 * p_ + i
                        nsl = slice(s * MT, (s + 1) * MT)
                        for kc in range(2):
                            nc.tensor.matmul(
                                ps[32 * i:32 * (i + 1), :],
                                wk_sb[d][:, kc, :], kkv[:, kc, nsl],
                                start=(kc == 0), stop=(kc == 1),
                                tile_position=(0, 32 * i))
                    nc.vector.tensor_scalar_add(
                        kt[d][:, p_ * MT:(p_ + 1) * MT], ps,
                        biasq_sb[:, 2 * d + 1:2 * d + 2])

                # V^T conv: vt[c] = f_kv[:, c*128:(c+1)*128]^T @ wv^T,
                # emitted in the order the attention groups consume the
                # chunks so the m-loop can start after the first four.
                vt_d = [None] * 32
                vt_order = [16 * (g >> 2) + 4 * i + (g & 3)
                            for g in range(NG) for i in range(4)]
                for c in vt_order:
                    ps = psM.tile([P, C], FP, tag="misc")
                    for kc in range(2):
                        nc.tensor.matmul(
                            ps, kkv[:, kc, c * P:(c + 1) * P],
                            wv_sb[d][:, kc, :],
                            start=(kc == 0), stop=(kc == 1))
                    v = vtp.tile([P, C], BF, tag="vt")
                    nc.vector.tensor_copy(v[:], ps)
                    vt_d[c] = v

                if d == 0:
                    emit_direct_conv()

                def emit_av_half(g, h, stg, av, vt_d=vt_d):
                    # half h covers key-bands i = 2h, 2h+1 of group g
                    p_, q_ = g >> 2, g & 3
                    for ii in range(2):
                        i = 2 * h + ii
                        c = 16 * p_ + 4 * i + q_
                        for cc in range(2):
                            nc.tensor.matmul(
                                av[cc], vt_d[c][:, cc * P:(cc + 1) * P],
                                stg[:, i, :],
                                start=(g == 0 and i == 0),
                                stop=(g == NG - 1 and i == 3))

                # ---- attention over m-tiles (software-pipelined) ----
                # PE queue order per group: S(g+1) is emitted BEFORE AV(g)
                # so exp(g+1) overlaps AV(g); the epilogue's PE ops are
                # deferred into the next m-tile's groups via `pending` so
                # their DVE-latency never head-of-line-blocks the PE queue.
                for t in range(NMT):
                    msl = slice(t * MT, (t + 1) * MT)
                    av = [psA.tile([P, MT], FP, tag="av", name=f"av{i}")
                          for i in range(2)]
                    racc = p_racc.tile([P, 4, MT], BF, tag="racc")
                    prev = None     # (g, stg) with AV not yet emitted
                    stg0 = None
                    for g in range(NG):
                        p_, q_ = g >> 2, g & 3
                        ksl = slice(p_ * MT + q_ * P, p_ * MT + (q_ + 1) * P)
                        # AV first half of the previous group, then the 4
                        # row-tiled K=32 score matmuls (concurrent on the
                        # 4 PE row-bands, into two 2-bank psum tiles).
                        if prev is not None:
                            emit_av_half(prev[0], 0, prev[1], av)
                        sps = [psS.tile([P, 2, MT], FP, tag="score",
                                        name=f"sps{h}")
                               for h in range(2)]
                        for i in range(4):
                            nc.tensor.matmul(
                                sps[i // 2][:, i % 2, :],
                                kt[d][32 * i:32 * (i + 1), ksl],
                                qr[d][32 * i:32 * (i + 1), msl],
                                start=True, stop=True,
                                tile_position=(32 * i, 0))
                        # exp halves (fp32 psum -> bf16 sbuf, into one
                        # stg tile) + a single [128,2048] rowsum add
                        stg = stp.tile([P, 4, MT], BF, tag="st")
                        for h in range(2):
                            nc.scalar.activation(stg[:, 2 * h:2 * h + 2, :],
                                                 sps[h][:, :, :], _ACT.Exp)
                        if g == 0:
                            stg0 = stg
                        elif g == 1:
                            nc.vector.tensor_add(racc[:], stg0[:], stg[:])
                        else:
                            nc.vector.tensor_add(racc[:], racc[:], stg[:])
                        # deferred epilogue piece of the previous m-tile
                        if pending:
                            pending.pop(0)()
                        if prev is not None:
                            emit_av_half(prev[0], 1, prev[1], av)
                        prev = (g, stg)
                    emit_av_half(prev[0], 0, prev[1], av)
                    emit_av_half(prev[0], 1, prev[1], av)

                    # rowsum fold + PE partition-fold + fast reciprocal;
                    # av -> sbuf immediately so the psum banks free up.
                    t0 = p_tmp.tile([P, MT], BF, tag="tmp")
                    t1 = p_tmp.tile([P, MT], BF, tag="tmp")
                    nc.vector.tensor_add(t0[:], racc[:, 0, :],
                                         racc[:, 1, :])
                    nc.vector.tensor_add(t1[:], racc[:, 2, :],
                                         racc[:, 3, :])
                    nc.vector.tensor_add(t0[:], t0[:], t1[:])
                    avs = []
                    for cc in range(2):
                        a = p_avsb.tile([P, MT], BF, tag="avsb")
                        nc.vector.tensor_copy(a[:], av[cc])
                        avs.append(a)
                    rsum_ps = psM.tile([1, MT], FP, tag="misc")
                    nc.tensor.matmul(rsum_ps, ones_col[:], t0[:],
                                     start=True, stop=True)
                    rinv = p_rinv.tile([1, MT], FP, tag="rinv")
                    rinv_bf = p_rinv.tile([1, MT], BF, tag="rinv")
                    rbc = p_rbc.tile([P, MT], BF, tag="rbc")
                    with nc.allow_low_precision(
                            reason="~51-ULP 1/rowsum at bf16; well inside "
                                   "the 2e-2 output gate"):
                        nc.vector.reciprocal_approx_fast(rinv[:], rsum_ps[:])
                        nc.vector.tensor_copy(rinv_bf[:], rinv[:])
                    # broadcast 1/rowsum to all partitions on idle GPSIMD
                    nc.gpsimd.partition_broadcast(rbc[:], rinv_bf[:])

                    def _stage1(avs=avs, rbc=rbc):
                        for cc in range(2):
                            nc.vector.tensor_mul(avs[cc][:], avs[cc][:],
                                                 rbc[:])

                    def _stage2(d=d, msl=msl, avs=avs):
                        yc = psM.tile([P, MT], FP, tag="misc")
                        nc.tensor.matmul(yc, wout_sb[:, 4 + 2 * d, 0:P],
                                         avs[0][:], start=True, stop=False)
                        nc.tensor.matmul(yc, wout_sb[:, 5 + 2 * d, 0:P],
                                         avs[1][:], start=False, stop=True)
                        nc.vector.tensor_add(y_acc[0][:, msl],
                                             y_acc[0][:, msl], yc)

                    def _stage3(d=d, t=t, msl=msl, avs=avs):
                        yc = psM.tile([P, MT], FP, tag="misc")
                        nc.tensor.matmul(yc, wout_sb[:, 4 + 2 * d, P:C],
                                         avs[0][:], start=True, stop=False)
                        nc.tensor.matmul(yc, wout_sb[:, 5 + 2 * d, P:C],
                                         avs[1][:], start=False, stop=True)
                        nc.vector.tensor_add(y_acc[1][:, msl],
                                             y_acc[1][:, msl], yc)
                        if d == 1:
                            nc.vector.bn_stats(bnacc[0][:, t, :],
                                               y_acc[0][:, msl])
                            nc.vector.bn_stats(bnacc[1][:, t, :],
                                               y_acc[1][:, msl])

                    pending.extend([_stage1, _stage2, _stage3])

            while pending:
                pending.pop(0)()

            # ---- BN: aggregate local stats, AllReduce, normalize ----
            # preload the sqrt activation table while the collective runs
            sq_dummy = p_small.tile([P, 1], FP, tag="bnm")
            nc.scalar.activation(sq_dummy[:], biasq_sb[:, 0:1], _ACT.Sqrt)

            stats = p_small.tile([P, 4], FP, tag="stats")
            for cc in range(2):
                mv = p_small.tile([P, 2], FP, tag="mv")
                nc.vector.bn_aggr(mv[:], bnacc[cc][:, :, :])
                # sum = mean * M ; sumsq = (var + mean^2) * M
                nc.vector.tensor_scalar_mul(stats[:, cc:cc + 1],
                                            mv[:, 0:1], float(M))
                sq = p_small.tile([P, 1], FP, tag="mv")
                nc.vector.tensor_tensor(sq[:], mv[:, 0:1], mv[:, 0:1],
                                        _ALU.mult)
                nc.vector.tensor_add(sq[:], sq[:], mv[:, 1:2])
                nc.vector.tensor_scalar_mul(stats[:, 2 + cc:3 + cc],
                                            sq[:], float(M))
            cc_in = dram.tile([P, 4], FP)
            cc_out = dram.tile([P, 4], FP)
            nc.sync.dma_start(cc_in[:], stats[:])
            nc.gpsimd.collective_compute(
                "AllReduce", _ALU.add,
                replica_groups=[list(range(NCORES))],
                ins=[cc_in.opt()], outs=[cc_out.opt()])
            ar = p_small.tile([P, 4], FP, tag="ar")
            nc.sync.dma_start(ar[:], cc_out[:])

            inv_n = 1.0 / BN_COUNT
            yo = yout.rearrange("(o p) m -> p o m", p=P)
            for cc in range(2):
                mean = p_small.tile([P, 1], FP, tag="bnm")
                ex2 = p_small.tile([P, 1], FP, tag="bnm")
                var = p_small.tile([P, 1], FP, tag="bnm")
                nc.vector.tensor_scalar_mul(mean[:], ar[:, cc:cc + 1], inv_n)
                nc.vector.tensor_scalar_mul(ex2[:], ar[:, 2 + cc:3 + cc],
                                            inv_n)
                nc.vector.tensor_tensor(var[:], mean[:], mean[:], _ALU.mult)
                nc.vector.tensor_sub(var[:], ex2[:], var[:])
                nc.vector.tensor_scalar_add(var[:], var[:], BN_EPS)
                sd = p_small.tile([P, 1], FP, tag="bnm")
                nc.scalar.activation(sd[:], var[:], _ACT.Sqrt)
                rstd = p_small.tile([P, 1], FP, tag="bnm")
                nc.vector.reciprocal(rstd[:], sd[:])
                scale = p_small.tile([P, 1], FP, tag="bnm")
                nc.vector.tensor_tensor(scale[:], cvec_sb[:, cc:cc + 1],
                                        rstd[:], _ALU.mult)
                shift = p_small.tile([P, 1], FP, tag="bnm")
                nc.vector.tensor_tensor(shift[:], mean[:], scale[:],
                                        _ALU.mult)
                nc.vector.tensor_sub(shift[:], cvec_sb[:, 2 + cc:3 + cc],
                                     shift[:])
                # normalize into bf16 staging + write back in 1024-wide
                # chunks so the DMA overlaps the next chunk's normalize
                # (bf16 output halves the writeback bytes; the host's
                # assembly casts back to fp32)
                for q in range(2):
                    qsl = slice(q * 1024, (q + 1) * 1024)
                    ybf = stp.tile([P, 1024], BF, tag="st", name="ybf")
                    nc.vector.tensor_scalar(
                        out=ybf[:], in0=y_acc[cc][:, qsl],
                        scalar1=scale[:], scalar2=shift[:],
                        op0=_ALU.mult, op1=_ALU.add)
                    nc.sync.dma_start(yo[:, cc, qsl], ybf[:])

    nc.compile()
    return nc


def _get_program():
    global _PROGRAM
    if _PROGRAM is None:
        _PROGRAM = _build_program()
    return _PROGRAM


def _make_in_maps(inputs):
    bf = ml_dtypes.bfloat16
    f_p = np.asarray(inputs["f_p"], np.float32).reshape(4, C, N)
    f_v = np.asarray(inputs["f_v"], np.float32).reshape(4, C, N)

    def T(x):
        return np.ascontiguousarray(
            np.asarray(x, np.float32).T.astype(bf))

    # direction 0 (p2v): q from f_p, k/v from f_v; dir 1 (v2p): reversed.
    shared = {
        "wq0": T(inputs["wq_p"]), "wk0": T(inputs["wk_v"]),
        "wv0": T(inputs["wv_v"]),
        "wq1": T(inputs["wq_v"]), "wk1": T(inputs["wk_p"]),
        "wv1": T(inputs["wv_p"]),
        "wout": T(inputs["w_out"]),
        "biasq": np.ascontiguousarray(np.stack(
            [np.tile(np.asarray(inputs[k], np.float32), 4)
             for k in ("bq_p", "bk_v", "bq_v", "bk_p")], axis=1)),
        "cvec": np.ascontiguousarray(np.stack(
            [np.asarray(inputs["gamma"], np.float32)[:P],
             np.asarray(inputs["gamma"], np.float32)[P:],
             np.asarray(inputs["beta"], np.float32)[:P],
             np.asarray(inputs["beta"], np.float32)[P:]], axis=1)),
    }
    in_maps = []
    for core in range(NCORES):
        b, h = divmod(core, 2)
        # roll so this core's query half sits at columns [0, 2048); K/V
        # use the full (permuted) range -- softmax/AV are key-order
        # invariant.
        kv1 = np.ascontiguousarray(
            np.roll(f_p[b], -h * M, axis=1).astype(bf))
        kv0 = np.ascontiguousarray(
            np.roll(f_v[b], -h * M, axis=1).astype(bf))
        in_maps.append({"kv0": kv0, "kv1": kv1, **shared})
    return in_maps


def _assemble(results):
    out = np.empty((4, C, N), np.float32)
    for core in range(NCORES):
        b, h = divmod(core, 2)
        out[b][:, h * M:(h + 1) * M] = np.asarray(
            results[core]["y"], np.float32)
    return out.reshape(4, C, 64, 64)


def _run(inputs, **kwargs):
    nc = _get_program()
    in_maps = _make_in_maps(inputs)
    res = bass_utils.run_bass_kernel_spmd(
        nc, in_maps, core_ids=list(range(NCORES)), **kwargs)
    return _assemble(res.results), res


def kernel(**inputs):
    out, _ = _run(inputs)
    return out
